# revision 12
# baseline (speedup 1.0000x reference)
"""Trainium2 Bass kernel for nn_Cand_50388556317014 (dense_cnn).

Computes, for x [16,16,16,128,128]:
    s    = sum_d x                         [B,16,H,W]
    inp  = [s, -x[:, :, 0], -x[:, :, -1]]  [B,48,H,W]
    y    = conv2d(inp, concat(wsum,wfront,wback), pad=1) + bias
    out  = tanh(softmax(y, ch))[:, :, None]

Sharding: data-parallel over batch (2 per core x 8 cores), weights replicated.

Per-core pipeline (per batch):
  1. DMA x depth-slices HBM->SBUF wide layout with accum_op=add (SWDGE CCE)
     into 4 accumulator chains; VectorE adds the 4 partials -> s.
  2. Scatter s into a row-padded image buffer buf[96p, 130x128] (pitch 128,
     8KB descriptors): partitions 0-47 = [s, x0, x15] (weights pre-negated
     on host), partitions 48-95 a one-row-shifted copy (for contraction
     packing of the dh taps).
  3. Conv: 6 float32r matmuls per 512-pixel tile (contraction 96/48),
     accumulated in PSUM. The pitch-128 layout makes the dw=+-1 taps wrap
     across row boundaries at output columns 0/127; 4 batch-wide correction
     matmuls compute the cancellation terms, applied per-tile by a tiny
     VectorE add.
  4. Channel softmax: PE-transpose logits (fp16) to [pixels, ch],
     reduce/exp with free-dim ops, tanh (bf16), PE-transpose back, DMA out.
"""

import sys

sys.path.insert(0, "/opt/trn_rl_repo")

import numpy as np

import concourse.bacc as bacc
import concourse.bass as bass
import concourse.tile as tile
from concourse import mybir
from concourse.bass_utils import run_bass_kernel_spmd

B, CIN, D, H, W = 16, 16, 16, 128, 128
COUT = 64
NCORES = 8
BL = B // NCORES          # batches per core
HP = H + 2                # 130 rows incl top/bottom pad
NBUF = HP * W + 2         # flat buffer: lead pad elem + 130x128 + tail pad
HW = H * W                # 16384
TILE_PX = 512             # output pixels per PSUM tile (4 rows)
NTILES = HW // TILE_PX    # 32
NCHUNK = TILE_PX // 128   # 4 transpose chunks per tile

f32 = mybir.dt.float32
f32r = mybir.dt.float32r
f16 = mybir.dt.float16
bf16 = mybir.dt.bfloat16
AF = mybir.ActivationFunctionType
ALU = mybir.AluOpType

# depth-sum: two chains of slice PAIRS. each pair = one plain DMA + one
# accum DMA (SWDGE CCE) into a stage tile; engines fold stages into the
# chain accumulator. chain A starts with d=0 and chain B with d=15 so the
# x0/x15 slices can be scattered out before accumulation clobbers them.
CHAIN_A = [(0, 8), (2, 10), (4, 12), (6, 14)]
CHAIN_B = [(15, 7), (1, 9), (3, 11), (5, 13)]


def _build_program():
    nc = bacc.Bacc("TRN2", target_bir_lowering=False, debug=False)

    x_dram = nc.dram_tensor("x", [BL, CIN, D, H, W], f32r, kind="ExternalInput")
    out_dram = nc.dram_tensor("out", [BL, COUT, HW], f32, kind="ExternalOutput")
    # weight packs (pre-transposed on host): lA{dw} [96,64] covers taps
    # (dh=0, dw) rows 0-47 and (dh=1, dw) rows 48-95; lB{dw} [48,64] = (dh=2, dw)
    lA_dram = [nc.dram_tensor(f"lA{dw}", [96, COUT], f32r, kind="ExternalInput") for dw in range(3)]
    lB_dram = [nc.dram_tensor(f"lB{dw}", [48, COUT], f32r, kind="ExternalInput") for dw in range(3)]
    # negated dw=0 / dw=2 packs for the column-edge wrap corrections
    lAn_dram = {dw: nc.dram_tensor(f"lA{dw}n", [96, COUT], f32r, kind="ExternalInput") for dw in (0, 2)}
    lBn_dram = {dw: nc.dram_tensor(f"lB{dw}n", [48, COUT], f32r, kind="ExternalInput") for dw in (0, 2)}
    bias_dram = nc.dram_tensor("bias", [COUT, 1], f32, kind="ExternalInput")
    ident16_dram = nc.dram_tensor("ident16", [128, 128], f16, kind="ExternalInput")
    identb_dram = nc.dram_tensor("identb", [128, 128], bf16, kind="ExternalInput")

    with tile.TileContext(nc) as tc:
        with (
            tc.tile_pool(name="consts", bufs=1) as consts,
            tc.tile_pool(name="bufp", bufs=1) as bufpool,
            tc.tile_pool(name="wide", bufs=1) as widepool,
            tc.tile_pool(name="logits", bufs=3) as logitpool,
            tc.tile_pool(name="soft", bufs=3) as softpool,
            tc.tile_pool(name="stats", bufs=4) as statpool,
            tc.tile_pool(name="outsb", bufs=3) as outpool,
            tc.tile_pool(name="psA", bufs=2, space="PSUM") as psumA,
            tc.tile_pool(name="psB", bufs=2, space="PSUM") as psumB,
            tc.tile_pool(name="psC", bufs=2, space="PSUM") as psumC,
            tc.tile_pool(name="psCorr", bufs=1, space="PSUM") as psumCorr,
        ):
            # ---- constants into SBUF ----
            lA = [consts.tile([96, COUT], f32r, tag=f"lA{dw}", name=f"lA{dw}_sb") for dw in range(3)]
            lB = [consts.tile([48, COUT], f32r, tag=f"lB{dw}", name=f"lB{dw}_sb") for dw in range(3)]
            lAn = {dw: consts.tile([96, COUT], f32r, tag=f"lA{dw}n", name=f"lA{dw}n_sb") for dw in (0, 2)}
            lBn = {dw: consts.tile([48, COUT], f32r, tag=f"lB{dw}n", name=f"lB{dw}n_sb") for dw in (0, 2)}
            bias_sb = consts.tile([COUT, 1], f32, tag="bias")
            ident16 = consts.tile([128, 128], f16, tag="ident16")
            identb = consts.tile([128, 128], bf16, tag="identb")
            for dw in range(3):
                nc.sync.dma_start(out=lA[dw][:], in_=lA_dram[dw][:])
                nc.sync.dma_start(out=lB[dw][:], in_=lB_dram[dw][:])
            for dw in (0, 2):
                nc.sync.dma_start(out=lAn[dw][:], in_=lAn_dram[dw][:])
                nc.sync.dma_start(out=lBn[dw][:], in_=lBn_dram[dw][:])
            nc.sync.dma_start(out=bias_sb[:], in_=bias_dram[:])
            nc.sync.dma_start(out=ident16[:], in_=ident16_dram[:])
            nc.sync.dma_start(out=identb[:], in_=identb_dram[:])

            # ---- padded image buffers, one per batch ----
            # flat layout: P[r, c] = buf[:, 1 + 128*r + c], r in [0,130).
            # rows 0/129 are zero pads; one lead + one tail pad element
            # absorb the out-of-range dw reads at the image corners.
            bufP = [bufpool.tile([96, NBUF], f32r, tag=f"bufP{i}", name=f"bufP{i}") for i in range(BL)]
            for i in range(BL):
                # interior rows + the shifted copy (48-95) are fully
                # rewritten every batch; only the pads need zeroing.
                nc.vector.memset(bufP[i][0:48, 0:1 + W].bitcast(f32), 0.0)
                nc.vector.memset(bufP[i][0:48, 1 + (HP - 1) * W:NBUF].bitcast(f32), 0.0)

            for b in range(BL):
                buf = bufP[b]

                # ---- 1. depth-sum: paired DMA-accum + engine adds ----
                def wide_src(d):
                    return x_dram[b, :, d, :, :].rearrange(
                        "c (hb r) w -> c hb (r w)", hb=8
                    ).transpose([1, 0, 2])

                acc = [widepool.tile([128, 2048], f32r, tag=f"acc{q}", name=f"acc{q}_{b}") for q in range(2)]
                stg = [widepool.tile([128, 2048], f32r, tag=f"stg{q}", name=f"stg{q}_{b}") for q in range(2)]
                # chain heads straight into the accumulators
                nc.sync.dma_start(out=acc[0][:], in_=wide_src(CHAIN_A[0][0]))
                nc.sync.dma_start(out=acc[1][:], in_=wide_src(CHAIN_B[0][0]))
                # x0 / x15 into canonical layout before accumulation
                for src_w, p0 in ((acc[0], 16), (acc[1], 32)):
                    for hb in range(8):
                        nc.sync.dma_start(
                            out=buf[p0:p0 + 16, 1 + (1 + 16 * hb) * W:1 + (17 + 16 * hb) * W],
                            in_=src_w[16 * hb:16 * (hb + 1), :],
                        )
                nc.gpsimd.dma_start(out=acc[0][:], in_=wide_src(CHAIN_A[0][1]), accum_op=ALU.add)
                nc.gpsimd.dma_start(out=acc[1][:], in_=wide_src(CHAIN_B[0][1]), accum_op=ALU.add)
                for j in (1, 2, 3):
                    for q, chain in ((0, CHAIN_A), (1, CHAIN_B)):
                        dp, da = chain[j]
                        nc.sync.dma_start(out=stg[q][:], in_=wide_src(dp))
                        nc.gpsimd.dma_start(out=stg[q][:], in_=wide_src(da), accum_op=ALU.add)
                    nc.vector.tensor_add(acc[0][:], acc[0][:], stg[0][:])
                    nc.gpsimd.tensor_add(acc[1][:], acc[1][:], stg[1][:])
                nc.vector.tensor_add(acc[0][:], acc[0][:], acc[1][:])

                # ---- 2. scatter s into canonical, then build shifted copy ----
                for hb in range(8):
                    nc.sync.dma_start(
                        out=buf[0:16, 1 + (1 + 16 * hb) * W:1 + (17 + 16 * hb) * W],
                        in_=acc[0][16 * hb:16 * (hb + 1), :],
                    )
                # partitions 48-95 = partitions 0-47 advanced by one row
                nc.sync.dma_start(
                    out=buf[48:96, 0:NBUF - W - 1],
                    in_=buf[0:48, W:NBUF - 1],
                )

                # ---- 2b. column-edge wrap corrections (whole batch) ----
                # main conv reads col -1 / col 128 as the wrapped neighbor-row
                # values; these 4 matmuls (negated dw=0 / dw=2 weights)
                # compute the cancellation, added per-tile below.
                # corr[:, 0, r] fixes out(r, 0); corr[:, 1, r] fixes out(r, 127).
                corr = psumCorr.tile([COUT, 2, H], f32, name=f"corr_{b}")

                def col_view(p_hi, base, nrows):
                    v = buf[0:p_hi, base:base + nrows * W]
                    return v.rearrange("p (r w) -> p r w", w=W)[:, :, 0:1]

                crhs = [
                    # out(r,0) reads P[r-1,127] (A) / P[r+1,127] (B)
                    (lAn[0], col_view(96, 0, H), corr[:, 0, :]),
                    (lBn[0], col_view(48, 2 * W, H), corr[:, 0, :]),
                    # out(r,127) reads P[r+1,0] (A) / P[r+3,0] (B; rows
                    # 126-127 read pad zeros, nothing to cancel)
                    (lAn[2], col_view(96, W + 1, H), corr[:, 1, :]),
                    (lBn[2], col_view(48, 3 * W + 1, H - 2), corr[:, 1, 0:H - 2]),
                ]
                for i, (lt, rhs, out_ap) in enumerate(crhs):
                    nc.tensor.matmul(
                        out_ap.unsqueeze(2),
                        lt[:],
                        rhs,
                        start=(i == 0),
                        stop=(i == 3),
                    )
                corr_sb = logitpool.tile([COUT, 2, H], f32, tag="corr_sb", name=f"corr_sb_{b}")
                nc.vector.tensor_copy(corr_sb[:], corr[:])

                # ---- 3+4. conv + softmax per 512-pixel tile ----
                ob = None
                for t in range(NTILES):
                    r0 = 4 * t
                    psA = psumA.tile([COUT, NCHUNK, 128], f32)
                    mms = []
                    for dw in range(3):
                        # (dh=0 via p0-47) + (dh=1 via shifted copy p48-95)
                        mms.append((lA[dw], buf[0:96, r0 * W + dw:r0 * W + dw + TILE_PX]))
                        # dh=2 via p0-47 two rows down
                        mms.append((lB[dw], buf[0:48, (r0 + 2) * W + dw:(r0 + 2) * W + dw + TILE_PX]))
                    for i, (lt, rhs) in enumerate(mms):
                        nc.tensor.matmul(
                            psA[:],
                            lt[:],
                            rhs.rearrange("p (a b) -> p a b", a=NCHUNK),
                            start=(i == 0),
                            stop=(i == len(mms) - 1),
                        )
                    # cancel the column-edge wrap terms on psA cols 0 / 127
                    e0 = psA[:, :, 0:1]
                    nc.vector.tensor_add(
                        e0, e0, corr_sb[:, 0, r0:r0 + 4].unsqueeze(2)
                    )
                    e1 = psA[:, :, 127:128]
                    nc.vector.tensor_add(
                        e1, e1, corr_sb[:, 1, r0:r0 + 4].unsqueeze(2)
                    )

                    # logits -> SBUF fp16 with bias add
                    st = logitpool.tile([COUT, TILE_PX], f16, tag="st")
                    nc.scalar.activation(
                        st[:],
                        psA[:].rearrange("p a b -> p (a b)"),
                        AF.Identity,
                        bias=bias_sb[:],
                    )

                    # transpose to [pixels, ch]
                    psB = psumB.tile([128, NCHUNK, COUT], f16)
                    for c in range(NCHUNK):
                        nc.tensor.transpose(
                            psB[:, c, :],
                            st[:, 128 * c:128 * (c + 1)],
                            ident16[0:64, 0:64],
                        )

                    # stable softmax + tanh in transposed layout
                    m = statpool.tile([128, NCHUNK], f32, tag="m")
                    nc.vector.reduce_max(m[:], psB[:], axis=mybir.AxisListType.X)
                    zc = softpool.tile([128, NCHUNK, COUT], f32, tag="zc")
                    nc.vector.tensor_sub(
                        zc[:], psB[:], m[:].unsqueeze(2).broadcast_to((128, NCHUNK, COUT))
                    )
                    ez = softpool.tile([128, NCHUNK, COUT], f32, tag="ez")
                    nc.scalar.activation(ez[:], zc[:], AF.Exp)
                    ssum = statpool.tile([128, NCHUNK], f32, tag="ssum")
                    nc.vector.reduce_sum(ssum[:], ez[:], axis=mybir.AxisListType.X)
                    rec = statpool.tile([128, NCHUNK], f32, tag="rec")
                    nc.vector.reciprocal(rec[:], ssum[:])
                    prob = softpool.tile([128, NCHUNK, COUT], f32, tag="prob")
                    nc.gpsimd.tensor_mul(
                        prob[:], ez[:], rec[:].unsqueeze(2).broadcast_to((128, NCHUNK, COUT))
                    )
                    th = softpool.tile([128, NCHUNK, COUT], bf16, tag="th")
                    nc.scalar.activation(th[:], prob[:], AF.Tanh)

                    # transpose back to [ch, pixels]; stage 2 tiles per out-DMA
                    psC = psumC.tile([COUT, TILE_PX], bf16)
                    for c in range(NCHUNK):
                        nc.tensor.transpose(
                            psC[:, 128 * c:128 * (c + 1)],
                            th[:, c, :],
                            identb[:],
                        )
                    if t % 2 == 0:
                        ob = outpool.tile([COUT, 2 * TILE_PX], f32, tag="ob")
                    half = ob[:, (t % 2) * TILE_PX:((t % 2) + 1) * TILE_PX]
                    if t % 2 == 0:
                        nc.vector.tensor_copy(half, psC[:])
                    else:
                        nc.scalar.copy(half, psC[:])
                        nc.sync.dma_start(
                            out=out_dram[b, :, TILE_PX * (t - 1):TILE_PX * (t + 1)],
                            in_=ob[:],
                        )

    nc.compile()
    return nc


def _prep_weights(wsum, wfront, wback, bias):
    w48 = np.concatenate(
        [wsum, -wfront, -wback], axis=1
    ).astype(np.float32)  # [64, 48, 3, 3]
    ins = {}
    for dw in range(3):
        ins[f"lA{dw}"] = np.ascontiguousarray(
            np.concatenate([w48[:, :, 0, dw].T, w48[:, :, 1, dw].T], axis=0)
        )
        ins[f"lB{dw}"] = np.ascontiguousarray(w48[:, :, 2, dw].T)
    for dw in (0, 2):
        ins[f"lA{dw}n"] = np.ascontiguousarray(-ins[f"lA{dw}"])
        ins[f"lB{dw}n"] = np.ascontiguousarray(-ins[f"lB{dw}"])
    ins["bias"] = np.ascontiguousarray(bias.astype(np.float32).reshape(COUT, 1))
    ins["ident16"] = np.eye(128, dtype=np.float16)
    import ml_dtypes
    ins["identb"] = np.eye(128).astype(ml_dtypes.bfloat16)
    return ins


_CACHE = {}


def _run(x, wsum, wfront, wback, bias, trace=False):
    if "nc" not in _CACHE:
        _CACHE["nc"] = _build_program()
    nc = _CACHE["nc"]

    x = np.ascontiguousarray(np.asarray(x, dtype=np.float32))
    wins = _prep_weights(
        np.asarray(wsum, np.float32),
        np.asarray(wfront, np.float32),
        np.asarray(wback, np.float32),
        np.asarray(bias, np.float32),
    )
    in_maps = []
    for i in range(NCORES):
        m = {"x": np.ascontiguousarray(x[BL * i:BL * (i + 1)])}
        m.update(wins)
        in_maps.append(m)

    res = run_bass_kernel_spmd(
        nc, in_maps, core_ids=list(range(NCORES)), trace=trace
    )
    out = np.concatenate([r["out"] for r in res.results], axis=0)
    out = out.reshape(B, COUT, 1, H, W).astype(np.float32)
    return out, res


def kernel(x, wsum, wfront, wback, bias):
    out, _ = _run(x, wsum, wfront, wback, bias, trace=False)
    return out


# revision 15
# speedup vs baseline: 1.0732x; 1.0732x over previous
"""Trainium2 Bass kernel for nn_Cand_50388556317014 (dense_cnn).

Computes, for x [16,16,16,128,128]:
    s    = sum_d x                         [B,16,H,W]
    inp  = [s, -x[:, :, 0], -x[:, :, -1]]  [B,48,H,W]
    y    = conv2d(inp, concat(wsum,wfront,wback), pad=1) + bias
    out  = tanh(softmax(y, ch))[:, :, None]

Sharding: data-parallel over batch (2 per core x 8 cores), weights replicated.

Per-core pipeline (per batch):
  1. DMA x depth-slices HBM->SBUF wide layout with accum_op=add (SWDGE CCE)
     into 4 accumulator chains; VectorE adds the 4 partials -> s.
  2. Scatter s into a row-padded image buffer buf[96p, 130x128] (pitch 128,
     8KB descriptors): partitions 0-47 = [s, x0, x15] (weights pre-negated
     on host), partitions 48-95 a one-row-shifted copy (for contraction
     packing of the dh taps).
  3. Conv: 6 float32r matmuls per 512-pixel tile (contraction 96/48),
     accumulated in PSUM. The pitch-128 layout makes the dw=+-1 taps wrap
     across row boundaries at output columns 0/127; 4 batch-wide correction
     matmuls compute the cancellation terms, applied per-tile by a tiny
     VectorE add.
  4. Channel softmax: PE-transpose logits (fp16) to [pixels, ch],
     reduce/exp with free-dim ops, tanh (bf16), PE-transpose back, DMA out.
"""

import sys

sys.path.insert(0, "/opt/trn_rl_repo")

import numpy as np

import concourse.bacc as bacc
import concourse.bass as bass
import concourse.tile as tile
from concourse import mybir
from concourse.bass_utils import run_bass_kernel_spmd

B, CIN, D, H, W = 16, 16, 16, 128, 128
COUT = 64
NCORES = 8
BL = B // NCORES          # batches per core
HP = H + 2                # 130 rows incl top/bottom pad
NBUF = HP * W + 2         # flat buffer: lead pad elem + 130x128 + tail pad
HW = H * W                # 16384
TILE_PX = 512             # output pixels per PSUM tile (4 rows)
NTILES = HW // TILE_PX    # 32
NCHUNK = TILE_PX // 128   # 4 transpose chunks per tile

f32 = mybir.dt.float32
f32r = mybir.dt.float32r
f16 = mybir.dt.float16
bf16 = mybir.dt.bfloat16
AF = mybir.ActivationFunctionType
ALU = mybir.AluOpType

# depth-sum: two chains of slice PAIRS. each pair = one plain DMA + one
# accum DMA (SWDGE CCE) into a stage tile; engines fold stages into the
# chain accumulator. chain A starts with d=0 and chain B with d=15 so the
# x0/x15 slices can be scattered out before accumulation clobbers them.
CHAIN_A = [(0, 8), (2, 10), (4, 12), (6, 14)]
CHAIN_B = [(15, 7), (1, 9), (3, 11), (5, 13)]


def _build_program():
    nc = bacc.Bacc("TRN2", target_bir_lowering=False, debug=False)

    x_dram = nc.dram_tensor("x", [BL, CIN, D, H, W], f32r, kind="ExternalInput")
    out_dram = nc.dram_tensor("out", [BL, COUT, HW], f32, kind="ExternalOutput")
    # weight packs (pre-transposed on host): lA{dw} [96,64] covers taps
    # (dh=0, dw) rows 0-47 and (dh=1, dw) rows 48-95; lB{dw} [48,64] = (dh=2, dw)
    lA_dram = [nc.dram_tensor(f"lA{dw}", [96, COUT], f32r, kind="ExternalInput") for dw in range(3)]
    lB_dram = [nc.dram_tensor(f"lB{dw}", [48, COUT], f32r, kind="ExternalInput") for dw in range(3)]
    # negated dw=0 / dw=2 packs for the column-edge wrap corrections
    lAn_dram = {dw: nc.dram_tensor(f"lA{dw}n", [96, COUT], f32r, kind="ExternalInput") for dw in (0, 2)}
    lBn_dram = {dw: nc.dram_tensor(f"lB{dw}n", [48, COUT], f32r, kind="ExternalInput") for dw in (0, 2)}
    bias_dram = nc.dram_tensor("bias", [COUT, 1], f32, kind="ExternalInput")
    ident16_dram = nc.dram_tensor("ident16", [128, 128], f16, kind="ExternalInput")
    identb_dram = nc.dram_tensor("identb", [128, 128], bf16, kind="ExternalInput")

    with tile.TileContext(nc) as tc:
        with (
            tc.tile_pool(name="consts", bufs=1) as consts,
            tc.tile_pool(name="bufp", bufs=1) as bufpool,
            tc.tile_pool(name="wide", bufs=1) as widepool,
            tc.tile_pool(name="logits", bufs=3) as logitpool,
            tc.tile_pool(name="soft", bufs=3) as softpool,
            tc.tile_pool(name="stats", bufs=4) as statpool,
            tc.tile_pool(name="outsb", bufs=3) as outpool,
            tc.tile_pool(name="psA", bufs=3, space="PSUM") as psumA,
            tc.tile_pool(name="psB", bufs=2, space="PSUM") as psumB,
            tc.tile_pool(name="psC", bufs=2, space="PSUM") as psumC,
        ):
            # ---- constants into SBUF ----
            lA = [consts.tile([96, COUT], f32r, tag=f"lA{dw}", name=f"lA{dw}_sb") for dw in range(3)]
            lB = [consts.tile([48, COUT], f32r, tag=f"lB{dw}", name=f"lB{dw}_sb") for dw in range(3)]
            lAn = {dw: consts.tile([96, COUT], f32r, tag=f"lA{dw}n", name=f"lA{dw}n_sb") for dw in (0, 2)}
            lBn = {dw: consts.tile([48, COUT], f32r, tag=f"lB{dw}n", name=f"lB{dw}n_sb") for dw in (0, 2)}
            bias_sb = consts.tile([COUT, 1], f32, tag="bias")
            ident16 = consts.tile([128, 128], f16, tag="ident16")
            identb = consts.tile([128, 128], bf16, tag="identb")
            for dw in range(3):
                nc.sync.dma_start(out=lA[dw][:], in_=lA_dram[dw][:])
                nc.sync.dma_start(out=lB[dw][:], in_=lB_dram[dw][:])
            for dw in (0, 2):
                nc.sync.dma_start(out=lAn[dw][:], in_=lAn_dram[dw][:])
                nc.sync.dma_start(out=lBn[dw][:], in_=lBn_dram[dw][:])
            nc.sync.dma_start(out=bias_sb[:], in_=bias_dram[:])
            nc.sync.dma_start(out=ident16[:], in_=ident16_dram[:])
            nc.sync.dma_start(out=identb[:], in_=identb_dram[:])

            # ---- padded image buffers, one per batch ----
            # flat layout: P[r, c] = buf[:, 1 + 128*r + c], r in [0,130).
            # rows 0/129 are zero pads; one lead + one tail pad element
            # absorb the out-of-range dw reads at the image corners.
            bufP = [bufpool.tile([96, NBUF], f32r, tag=f"bufP{i}", name=f"bufP{i}") for i in range(BL)]
            for i in range(BL):
                # interior rows + the shifted copy (48-95) are fully
                # rewritten every batch; only the pads need zeroing.
                nc.vector.memset(bufP[i][0:48, 0:1 + W].bitcast(f32), 0.0)
                nc.vector.memset(bufP[i][0:48, 1 + (HP - 1) * W:NBUF].bitcast(f32), 0.0)

            # ================= PHASE 1: loads for all batches =============
            # emitted before any compute so batch b+1's loads are not queued
            # behind batch b's compute ops in the engine FIFOs.
            for b in range(BL):
                buf = bufP[b]

                # ---- 1. depth-sum: paired DMA-accum + engine adds ----
                def wide_src(d):
                    return x_dram[b, :, d, :, :].rearrange(
                        "c (hb r) w -> c hb (r w)", hb=8
                    ).transpose([1, 0, 2])

                acc = [widepool.tile([128, 2048], f32r, tag=f"acc{q}", name=f"acc{q}_{b}") for q in range(2)]
                stg = [widepool.tile([128, 2048], f32r, tag=f"stg{q}", name=f"stg{q}_{b}") for q in range(2)]
                # chain heads straight into the accumulators
                nc.sync.dma_start(out=acc[0][:], in_=wide_src(CHAIN_A[0][0]))
                nc.sync.dma_start(out=acc[1][:], in_=wide_src(CHAIN_B[0][0]))
                # x0 / x15 into canonical layout before accumulation
                for src_w, p0 in ((acc[0], 16), (acc[1], 32)):
                    for hb in range(8):
                        nc.sync.dma_start(
                            out=buf[p0:p0 + 16, 1 + (1 + 16 * hb) * W:1 + (17 + 16 * hb) * W],
                            in_=src_w[16 * hb:16 * (hb + 1), :],
                        )
                nc.gpsimd.dma_start(out=acc[0][:], in_=wide_src(CHAIN_A[0][1]), accum_op=ALU.add)
                nc.gpsimd.dma_start(out=acc[1][:], in_=wide_src(CHAIN_B[0][1]), accum_op=ALU.add)
                for j in (1, 2, 3):
                    for q, chain in ((0, CHAIN_A), (1, CHAIN_B)):
                        dp, da = chain[j]
                        nc.sync.dma_start(out=stg[q][:], in_=wide_src(dp))
                        nc.gpsimd.dma_start(out=stg[q][:], in_=wide_src(da), accum_op=ALU.add)
                    nc.vector.tensor_add(acc[0][:], acc[0][:], stg[0][:])
                    nc.gpsimd.tensor_add(acc[1][:], acc[1][:], stg[1][:])
                nc.vector.tensor_add(acc[0][:], acc[0][:], acc[1][:])

                # ---- 2. scatter s into canonical, then build shifted copy ----
                for hb in range(8):
                    nc.sync.dma_start(
                        out=buf[0:16, 1 + (1 + 16 * hb) * W:1 + (17 + 16 * hb) * W],
                        in_=acc[0][16 * hb:16 * (hb + 1), :],
                    )
                # partitions 48-95 = partitions 0-47 advanced by one row
                nc.sync.dma_start(
                    out=buf[48:96, 0:NBUF - W - 1],
                    in_=buf[0:48, W:NBUF - 1],
                )

            # ================= PHASE 2: compute per batch =================
            for b in range(BL):
                buf = bufP[b]

                # ---- 2b. column-edge wrap corrections (whole batch) ----
                # main conv reads col -1 / col 128 as the wrapped neighbor-row
                # values; these 4 matmuls (negated dw=0 / dw=2 weights)
                # compute the cancellation, added per-tile below.
                # corr[:, 0, r] fixes out(r, 0); corr[:, 1, r] fixes out(r, 127).
                corr = psumB.tile([COUT, 2, H], f32, bufs=1, tag="corr", name=f"corr_{b}")

                def col_view(p_hi, base, nrows):
                    v = buf[0:p_hi, base:base + nrows * W]
                    return v.rearrange("p (r w) -> p r w", w=W)[:, :, 0:1]

                crhs = [
                    # out(r,0) reads P[r-1,127] (A) / P[r+1,127] (B)
                    (lAn[0], col_view(96, 0, H), corr[:, 0, :]),
                    (lBn[0], col_view(48, 2 * W, H), corr[:, 0, :]),
                    # out(r,127) reads P[r+1,0] (A) / P[r+3,0] (B; rows
                    # 126-127 read pad zeros, nothing to cancel)
                    (lAn[2], col_view(96, W + 1, H), corr[:, 1, :]),
                    (lBn[2], col_view(48, 3 * W + 1, H - 2), corr[:, 1, 0:H - 2]),
                ]
                for i, (lt, rhs, out_ap) in enumerate(crhs):
                    nc.tensor.matmul(
                        out_ap.unsqueeze(2),
                        lt[:],
                        rhs,
                        start=(i == 0),
                        stop=(i == 3),
                    )
                corr_sb = logitpool.tile([COUT, 2, H], f32, tag="corr_sb", name=f"corr_sb_{b}")
                nc.vector.tensor_copy(corr_sb[:], corr[:])

                # ---- 3+4. conv + softmax per 512-pixel tile ----
                ob = None
                for t in range(NTILES):
                    r0 = 4 * t
                    psA = psumA.tile([COUT, NCHUNK, 128], f32)
                    mms = []
                    for dw in range(3):
                        # (dh=0 via p0-47) + (dh=1 via shifted copy p48-95)
                        mms.append((lA[dw], buf[0:96, r0 * W + dw:r0 * W + dw + TILE_PX]))
                        # dh=2 via p0-47 two rows down
                        mms.append((lB[dw], buf[0:48, (r0 + 2) * W + dw:(r0 + 2) * W + dw + TILE_PX]))
                    for i, (lt, rhs) in enumerate(mms):
                        nc.tensor.matmul(
                            psA[:],
                            lt[:],
                            rhs.rearrange("p (a b) -> p a b", a=NCHUNK),
                            start=(i == 0),
                            stop=(i == len(mms) - 1),
                        )
                    # cancel the column-edge wrap terms on psA cols 0 / 127
                    e0 = psA[:, :, 0:1]
                    nc.vector.tensor_add(
                        e0, e0, corr_sb[:, 0, r0:r0 + 4].unsqueeze(2)
                    )
                    e1 = psA[:, :, 127:128]
                    nc.vector.tensor_add(
                        e1, e1, corr_sb[:, 1, r0:r0 + 4].unsqueeze(2)
                    )

                    # logits -> SBUF fp16 with bias add
                    st = logitpool.tile([COUT, TILE_PX], f16, tag="st")
                    nc.scalar.activation(
                        st[:],
                        psA[:].rearrange("p a b -> p (a b)"),
                        AF.Identity,
                        bias=bias_sb[:],
                    )

                    # transpose to [pixels, ch]
                    psB = psumB.tile([128, NCHUNK, COUT], f16)
                    for c in range(NCHUNK):
                        nc.tensor.transpose(
                            psB[:, c, :],
                            st[:, 128 * c:128 * (c + 1)],
                            ident16[0:64, 0:64],
                        )

                    # stable softmax + tanh in transposed layout
                    m = statpool.tile([128, NCHUNK], f32, tag="m")
                    nc.vector.reduce_max(m[:], psB[:], axis=mybir.AxisListType.X)
                    zc = softpool.tile([128, NCHUNK, COUT], f32, tag="zc")
                    nc.vector.tensor_sub(
                        zc[:], psB[:], m[:].unsqueeze(2).broadcast_to((128, NCHUNK, COUT))
                    )
                    ez = softpool.tile([128, NCHUNK, COUT], f32, tag="ez")
                    nc.scalar.activation(ez[:], zc[:], AF.Exp)
                    ssum = statpool.tile([128, NCHUNK], f32, tag="ssum")
                    nc.vector.reduce_sum(ssum[:], ez[:], axis=mybir.AxisListType.X)
                    rec = statpool.tile([128, NCHUNK], f32, tag="rec")
                    nc.vector.reciprocal(rec[:], ssum[:])
                    prob = softpool.tile([128, NCHUNK, COUT], f32, tag="prob")
                    nc.gpsimd.tensor_mul(
                        prob[:], ez[:], rec[:].unsqueeze(2).broadcast_to((128, NCHUNK, COUT))
                    )
                    th = softpool.tile([128, NCHUNK, COUT], bf16, tag="th")
                    nc.scalar.activation(th[:], prob[:], AF.Tanh)

                    # transpose back to [ch, pixels]; stage 2 tiles per out-DMA
                    psC = psumC.tile([COUT, TILE_PX], bf16)
                    for c in range(NCHUNK):
                        nc.tensor.transpose(
                            psC[:, 128 * c:128 * (c + 1)],
                            th[:, c, :],
                            identb[:],
                        )
                    if t % 2 == 0:
                        ob = outpool.tile([COUT, 2 * TILE_PX], f32, tag="ob")
                    half = ob[:, (t % 2) * TILE_PX:((t % 2) + 1) * TILE_PX]
                    if t % 2 == 0:
                        nc.vector.tensor_copy(half, psC[:])
                    else:
                        nc.scalar.copy(half, psC[:])
                        nc.sync.dma_start(
                            out=out_dram[b, :, TILE_PX * (t - 1):TILE_PX * (t + 1)],
                            in_=ob[:],
                        )

    nc.compile()
    return nc


def _prep_weights(wsum, wfront, wback, bias):
    w48 = np.concatenate(
        [wsum, -wfront, -wback], axis=1
    ).astype(np.float32)  # [64, 48, 3, 3]
    ins = {}
    for dw in range(3):
        ins[f"lA{dw}"] = np.ascontiguousarray(
            np.concatenate([w48[:, :, 0, dw].T, w48[:, :, 1, dw].T], axis=0)
        )
        ins[f"lB{dw}"] = np.ascontiguousarray(w48[:, :, 2, dw].T)
    for dw in (0, 2):
        ins[f"lA{dw}n"] = np.ascontiguousarray(-ins[f"lA{dw}"])
        ins[f"lB{dw}n"] = np.ascontiguousarray(-ins[f"lB{dw}"])
    ins["bias"] = np.ascontiguousarray(bias.astype(np.float32).reshape(COUT, 1))
    ins["ident16"] = np.eye(128, dtype=np.float16)
    import ml_dtypes
    ins["identb"] = np.eye(128).astype(ml_dtypes.bfloat16)
    return ins


_CACHE = {}


def _run(x, wsum, wfront, wback, bias, trace=False):
    if "nc" not in _CACHE:
        _CACHE["nc"] = _build_program()
    nc = _CACHE["nc"]

    x = np.ascontiguousarray(np.asarray(x, dtype=np.float32))
    wins = _prep_weights(
        np.asarray(wsum, np.float32),
        np.asarray(wfront, np.float32),
        np.asarray(wback, np.float32),
        np.asarray(bias, np.float32),
    )
    in_maps = []
    for i in range(NCORES):
        m = {"x": np.ascontiguousarray(x[BL * i:BL * (i + 1)])}
        m.update(wins)
        in_maps.append(m)

    res = run_bass_kernel_spmd(
        nc, in_maps, core_ids=list(range(NCORES)), trace=trace
    )
    out = np.concatenate([r["out"] for r in res.results], axis=0)
    out = out.reshape(B, COUT, 1, H, W).astype(np.float32)
    return out, res


def kernel(x, wsum, wfront, wback, bias):
    out, _ = _run(x, wsum, wfront, wback, bias, trace=False)
    return out
